# revision 12
# baseline (speedup 1.0000x reference)
"""Multi-head causal attention (B=2,S=2048,D=1024,H=16) on 8 TRN2 NeuronCores.

Sharding: 8 cores = 2-way batch data-parallel x 4-way head tensor-parallel
(4 heads = 256 local dims per core). Each core computes q/k/v projections for
its head group, causal attention, and a partial out-projection (w_out
row-sharded). Host sums the 4 partials per batch element and adds b_out.

On-device layout (per core, bf16 compute, f32 accumulate):
  xT   [D, S]  = x[b].T                      (dram, bf16)
  qT/kT[256,S] = Wq_l @ x.T  (PE, K=D)       heads packed 64 rows each
  v    [S,260] = x @ WvT_l   (PE), 65-stride head interleave w/ ones column
  S.T  [k, q]  = k_h @ q_h.T (PE, K=64, two heads row-packed in PE array)
  P.T  = exp(S.T/8) (ACT, causal blocks only; diag blocks masked by DVE mul)
  outT_aug [65, q] = [v_h|1].T @ P.T  (PE, K=128) -> row 64 = softmax denom
  attnT = outT_aug[0:64] * bcast(1/denom)  (DVE + gpsimd partition_broadcast)
  yT_partial [D, S] = W_l @ attnT  (PE, K=256, interleaved per q-chunk)
"""

import sys

for _p in ("/opt/trn_rl_repo", "/root/.axon_site/_ro/trn_rl_repo"):
    if _p not in sys.path:
        sys.path.append(_p)

import numpy as np
import ml_dtypes

BF16 = ml_dtypes.bfloat16

B, S, D, H = 2, 2048, 1024, 16
HD = D // H            # 64
N_CORES = 8
TP = 4                 # head groups
HL = H // TP           # 4 heads per core
DL = HL * HD           # 256 local dims
VW = HD + 1            # 65: v columns + ones column

_BUILD_CACHE = {}


def build_nc(s=S, debug=False):
    """Build + finalize the Bacc graph for one core (SPMD across 8)."""
    from concourse import bacc
    import concourse.mybir as mybir
    import concourse.tile as tile

    bf = mybir.dt.bfloat16
    f32 = mybir.dt.float32
    Exp = mybir.ActivationFunctionType.Exp

    KT = s // 128          # k tiles
    QC = s // 512          # q chunks
    MT = DL // 128         # 2 (also: head pairs)
    DKT = D // 128         # 8 contraction tiles for projections

    nc = bacc.Bacc()
    xT_d = nc.declare_dram_parameter("xT", [D, s], bf, isOutput=False)
    wqT_d = nc.declare_dram_parameter("wqT", [D, DL], bf, isOutput=False)
    wkT_d = nc.declare_dram_parameter("wkT", [D, DL], bf, isOutput=False)
    wvT_d = nc.declare_dram_parameter("wvT", [D, DL], bf, isOutput=False)
    wo_d = nc.declare_dram_parameter("wo", [DL, D], bf, isOutput=False)
    mask_d = nc.declare_dram_parameter("mask", [128, 2048], bf, isOutput=False)
    bq_d = nc.declare_dram_parameter("bq", [MT, 128, 1], f32, isOutput=False)
    bk_d = nc.declare_dram_parameter("bk", [MT, 128, 1], f32, isOutput=False)
    bv_d = nc.declare_dram_parameter("bv", [128, HL * VW], f32, isOutput=False)
    yT_d = nc.declare_dram_parameter("yT", [D, s], f32, isOutput=True)
    if debug:
        dbg_q = nc.declare_dram_parameter("dbg_q", [128, DL // 128, s], f32, isOutput=True)
        dbg_k = nc.declare_dram_parameter("dbg_k", [128, DL // 128, s], f32, isOutput=True)
        dbg_v = nc.declare_dram_parameter("dbg_v", [128, s // 128, HL * VW], f32, isOutput=True)
        dbg_a = nc.declare_dram_parameter("dbg_a", [128, DL // 128, s], f32, isOutput=True)
        dbg_pt = nc.declare_dram_parameter("dbg_pt", [128, s // 128, 1024], f32, isOutput=True)
        dbg_aug = nc.declare_dram_parameter("dbg_aug", [128, 2, 512], f32, isOutput=True)
        dbg_bc = nc.declare_dram_parameter("dbg_bc", [128, 2, 512], f32, isOutput=True)

    with tile.TileContext(nc) as tc:
        with (
            tc.tile_pool(name="const", bufs=1) as cpool,
            tc.tile_pool(name="work", bufs=1) as wpool,
            tc.tile_pool(name="pt", bufs=2) as ppool,
            tc.tile_pool(name="norm", bufs=2) as npool,
            tc.tile_pool(name="ystage", bufs=4) as ypool,
            tc.tile_pool(name="qkv_ps", bufs=2, space="PSUM") as qkv_ps,
            tc.tile_pool(name="score_ps", bufs=1, space="PSUM") as score_ps,
            tc.tile_pool(name="aug_ps", bufs=2, space="PSUM") as aug_ps,
        ):
            # ---- constants / inputs to SBUF ----
            wq_sb = cpool.tile([128, DKT, DL], bf)
            nc.sync.dma_start(wq_sb[:], wqT_d.ap().rearrange("(kt p) n -> p kt n", p=128))
            wk_sb = cpool.tile([128, DKT, DL], bf)
            nc.sync.dma_start(wk_sb[:], wkT_d.ap().rearrange("(kt p) n -> p kt n", p=128))
            x_sb = cpool.tile([128, DKT, s], bf)
            for kt in range(DKT):
                nc.sync.dma_start(x_sb[:, kt, :], xT_d[kt * 128:(kt + 1) * 128, :])
            wv_sb = cpool.tile([128, DKT, DL], bf)
            nc.sync.dma_start(wv_sb[:], wvT_d.ap().rearrange("(kt p) n -> p kt n", p=128))
            wo_sb = cpool.tile([128, MT, D], bf)
            nc.sync.dma_start(wo_sb[:], wo_d.ap().rearrange("(kt p) n -> p kt n", p=128))
            mask_sb = cpool.tile([128, 4, 512], bf)
            nc.sync.dma_start(mask_sb[:], mask_d.ap().rearrange("p (t n) -> p t n", n=512))
            bq_sb = cpool.tile([128, MT, 1], f32)
            nc.sync.dma_start(bq_sb[:], bq_d.ap().rearrange("m p o -> p m o"))
            bk_sb = cpool.tile([128, MT, 1], f32)
            nc.sync.dma_start(bk_sb[:], bk_d.ap().rearrange("m p o -> p m o"))
            bvb_sb = cpool.tile([128, HL * VW], f32)
            nc.sync.dma_start(bvb_sb[:], bv_d.ap())
            ones_sb = cpool.tile([128, 64], f32)
            nc.vector.memset(ones_sb[64:65, :], 1.0)

            qT_sb = wpool.tile([128, MT, s], bf)
            kT_sb = wpool.tile([128, MT, s], bf)
            v_sb = wpool.tile([128, KT, HL * VW], bf)
            attnT_sb = wpool.tile([128, MT, s], bf)
            for kt in range(KT):
                ones_ap = v_sb[:, kt, :].rearrange("p (h x) -> p h x", x=VW)[:, :, HD:VW]
                nc.vector.memset(ones_ap, 1.0)

            # ---- projections: q/k for pair 0 first so attention starts early
            def project(dst, w_sb, b_sb, mt):
                for qc4 in range(s // 512):
                    ps = qkv_ps.tile([128, 512], f32, tag="qk", name="proj_ps")
                    for kt in range(DKT):
                        nc.tensor.matmul(
                            ps[:],
                            w_sb[:, kt, mt * 128:(mt + 1) * 128],
                            x_sb[:, kt, qc4 * 512:(qc4 + 1) * 512],
                            start=(kt == 0), stop=(kt == DKT - 1),
                        )
                    nc.vector.tensor_scalar_add(
                        dst[:, mt, qc4 * 512:(qc4 + 1) * 512], ps[:], b_sb[:, mt, :]
                    )

            project(qT_sb, wq_sb, bq_sb, 0)
            project(kT_sb, wk_sb, bk_sb, 0)
            project(qT_sb, wq_sb, bq_sb, 1)
            project(kT_sb, wk_sb, bk_sb, 1)
            for st_ in range(KT):
                ps = qkv_ps.tile([128, DL], f32, tag="qk", name="v_ps")
                for kt in range(DKT):
                    nc.tensor.matmul(
                        ps[:],
                        x_sb[:, kt, st_ * 128:(st_ + 1) * 128],
                        wv_sb[:, kt, :],
                        start=(kt == 0), stop=(kt == DKT - 1),
                    )
                nc.vector.tensor_add(
                    v_sb[:, st_, :].rearrange("p (h x) -> p h x", x=VW)[:, :, 0:HD],
                    ps[:].rearrange("p (h x) -> p h x", x=HD),
                    bvb_sb[:].rearrange("p (h x) -> p h x", x=VW)[:, :, 0:HD],
                )

            # ---- attention (qc outer) + interleaved out-projection ----
            for qc in range(QC):
                n_kt = 4 * qc + 4
                for p in range(MT):   # pair p: heads 2p (rows 0:64), 2p+1 (64:128)
                    pt = ppool.tile([128, KT, 1024], bf, tag="pt", name="pt")
                    aug0 = aug_ps.tile([128, 512], f32, tag="aug", name="aug0")
                    aug1 = aug_ps.tile([128, 512], f32, tag="aug", name="aug1")
                    for g in range(n_kt // 2):
                        st = score_ps.tile([128, 4, 512], f32, tag="st", name="st")
                        for j in (0, 1):
                            kt = 2 * g + j
                            nc.tensor.matmul(
                                st[:, j, :],
                                kT_sb[0:64, p, kt * 128:(kt + 1) * 128],
                                qT_sb[0:64, p, qc * 512:(qc + 1) * 512],
                            )
                            nc.tensor.matmul(
                                st[:, 2 + j, :],
                                kT_sb[64:128, p, kt * 128:(kt + 1) * 128],
                                qT_sb[64:128, p, qc * 512:(qc + 1) * 512],
                            )
                        # one FD=2048 exp covering both heads x two k-tiles:
                        # st slot (h*2+j) -> pt[2g+j, h*512 : h*512+512]
                        nc.scalar.activation(
                            pt[:, 2 * g:2 * g + 2, :].rearrange(
                                "p k (h n) -> p h k n", n=512
                            ),
                            st[:].rearrange("p (h k) n -> p h k n", h=2),
                            Exp, scale=0.125,
                        )
                        for j in (0, 1):
                            kt = 2 * g + j
                            t = kt - 4 * qc
                            if t >= 0:
                                w = 128 * (t + 1)
                                nc.vector.tensor_mul(
                                    pt[:, kt, 0:w], pt[:, kt, 0:w], mask_sb[:, t, 0:w]
                                )
                                nc.vector.tensor_mul(
                                    pt[:, kt, 512:512 + w], pt[:, kt, 512:512 + w],
                                    mask_sb[:, t, 0:w],
                                )
                    for kt in range(n_kt):
                        nc.tensor.matmul(
                            aug0[0:VW, :],
                            v_sb[:, kt, (2 * p) * VW:(2 * p) * VW + VW],
                            pt[:, kt, 0:512],
                            start=(kt == 0), stop=(kt == n_kt - 1),
                        )
                        nc.tensor.matmul(
                            aug1[0:VW, :],
                            v_sb[:, kt, (2 * p + 1) * VW:(2 * p + 1) * VW + VW],
                            pt[:, kt, 512:1024],
                            start=(kt == 0), stop=(kt == n_kt - 1),
                        )
                    if debug and p == 0 and qc == 0:
                        with tc.tile_pool(name="dbgp", bufs=1) as dpp:
                            tpt = dpp.tile([128, s // 128, 1024], f32, tag="tpt", name="tpt")
                            nc.vector.tensor_copy(tpt[:, 0:n_kt, :], pt[:, 0:n_kt, :])
                            nc.sync.dma_start(dbg_pt.ap(), tpt[:])
                            taug = dpp.tile([128, 2, 512], f32, tag="taug", name="taug")
                            nc.vector.tensor_copy(taug[:, 0, :], aug0[:])
                            nc.vector.tensor_copy(taug[:, 1, :], aug1[:])
                            nc.sync.dma_start(dbg_aug.ap(), taug[:])
                    for hh, aug in ((0, aug0), (1, aug1)):
                        # custom-DVE recip only works at base partition 0 on
                        # HW and cannot read PSUM: copy the denominator row to
                        # SBUF, broadcast it to partitions 0:64 with a K=1
                        # matmul, then reciprocal the broadcast at base 0.
                        row = npool.tile([128, 512], f32, tag="row", name="row")
                        bc = npool.tile([128, 512], f32, tag="bc", name="bc")
                        nc.vector.tensor_copy(row[64:65, :], aug[64:65, :])
                        bcp = qkv_ps.tile([128, 512], f32, tag="qk", name="bcp")
                        nc.tensor.matmul(
                            bcp[0:64, :], ones_sb[64:65, :], row[64:65, :]
                        )
                        nc.vector.tensor_copy(bc[0:64, :], bcp[0:64, :])
                        nc.vector.reciprocal_approx_fast(bc[0:64, :], bc[0:64, :])
                        if debug and p == 0 and qc == 0:
                            with tc.tile_pool(name="dbgb", bufs=1) as dpb:
                                tbc = dpb.tile([128, 512], f32, tag="tbc" + str(hh), name="tbc")
                                nc.vector.tensor_copy(tbc[:], bc[:])
                                nc.sync.dma_start(dbg_bc[:, hh, :], tbc[:])
                        if hh == 0:
                            nc.vector.tensor_mul(
                                attnT_sb[0:64, p, qc * 512:(qc + 1) * 512],
                                aug[0:64, :], bc[0:64, :],
                            )
                        else:
                            sh = npool.tile([128, 512], bf, tag="sh", name="sh")
                            nc.vector.tensor_mul(sh[0:64, :], aug[0:64, :], bc[0:64, :])
                            nc.sync.dma_start(
                                attnT_sb[64:128, p, qc * 512:(qc + 1) * 512],
                                sh[0:64, :],
                            )

                # out-projection for this q chunk (fills PE while ACT exps)
                for mt8 in range(D // 128):
                    ps = qkv_ps.tile([128, 512], f32, tag="qk", name="y_ps")
                    for kt2 in range(MT):
                        nc.tensor.matmul(
                            ps[:],
                            wo_sb[:, kt2, mt8 * 128:(mt8 + 1) * 128],
                            attnT_sb[:, kt2, qc * 512:(qc + 1) * 512],
                            start=(kt2 == 0), stop=(kt2 == MT - 1),
                        )
                    y_sb = ypool.tile([128, 512], f32, tag="y", name="y_sb")
                    if mt8 % 2 == 0:
                        nc.scalar.copy(y_sb[:], ps[:])
                    else:
                        nc.vector.tensor_copy(y_sb[:], ps[:])
                    nc.sync.dma_start(
                        yT_d[mt8 * 128:(mt8 + 1) * 128, qc * 512:(qc + 1) * 512],
                        y_sb[:],
                    )

        if debug:
            with tc.tile_pool(name="dbg", bufs=1) as dpool:
                for name, tsrc, dst in (("q", qT_sb, dbg_q), ("k", kT_sb, dbg_k),
                                         ("v", v_sb, dbg_v), ("a", attnT_sb, dbg_a)):
                    tmp = dpool.tile(list(tsrc.shape), f32, tag="dbg"+name, name="dbg"+name)
                    nc.vector.tensor_copy(tmp[:], tsrc[:])
                    nc.sync.dma_start(dst.ap(), tmp[:])

    nc.finalize()
    return nc


def _prep_inputs(x, w_q, b_q, w_k, b_k, w_v, b_v, w_out, s=S):
    """Per-core input shards (host-side)."""
    # causal mask tiles: mask[i, t*512+j] = 1.0 if (128*t + i) <= j else 0
    i = np.arange(128)[:, None]
    j = np.arange(512)[None, :]
    mask = np.concatenate(
        [((128 * t + i) <= j).astype(np.float32) for t in range(4)], axis=1
    ).astype(BF16)

    in_maps = []
    for c in range(N_CORES):
        b, g = divmod(c, TP)
        sl = slice(g * DL, (g + 1) * DL)
        bv_row = np.zeros((1, HL * VW), np.float32)
        for h in range(HL):
            bv_row[0, h * VW:h * VW + HD] = b_v[g * DL + h * HD: g * DL + (h + 1) * HD]
        bv_row = np.broadcast_to(bv_row, (128, HL * VW)).copy()
        in_maps.append({
            "xT": np.ascontiguousarray(x[b, :s].T).astype(BF16),
            "wqT": np.ascontiguousarray(w_q[sl].T).astype(BF16),
            "wkT": np.ascontiguousarray(w_k[sl].T).astype(BF16),
            "wvT": np.ascontiguousarray(w_v[sl].T).astype(BF16),
            "wo": np.ascontiguousarray(w_out[:, sl].T).astype(BF16),
            "mask": mask,
            "bq": b_q[sl].reshape(DL // 128, 128, 1).astype(np.float32),
            "bk": b_k[sl].reshape(DL // 128, 128, 1).astype(np.float32),
            "bv": bv_row,
        })
    return in_maps


def _assemble(results, b_out, s=S):
    out = np.zeros((B, s, D), np.float32)
    for c in range(N_CORES):
        b = c // TP
        out[b] += results[c]["yT"].T
    out += b_out.astype(np.float32)
    return out


def kernel(x, w_q, b_q, w_k, b_k, w_v, b_v, w_out, b_out):
    from concourse.bass_utils import run_bass_kernel_spmd

    x = np.asarray(x, np.float32)
    if "nc" not in _BUILD_CACHE:
        _BUILD_CACHE["nc"] = build_nc(S)
    nc = _BUILD_CACHE["nc"]
    in_maps = _prep_inputs(
        x, np.asarray(w_q, np.float32), np.asarray(b_q, np.float32),
        np.asarray(w_k, np.float32), np.asarray(b_k, np.float32),
        np.asarray(w_v, np.float32), np.asarray(b_v, np.float32),
        np.asarray(w_out, np.float32), S,
    )
    res = run_bass_kernel_spmd(nc, in_maps, core_ids=list(range(N_CORES))).results
    return _assemble(res, np.asarray(b_out, np.float32), S)
